# revision 7
# baseline (speedup 1.0000x reference)
"""Boundary-point Chamfer loss on 8 Trainium2 NeuronCores.

Math: pts = img_render_points[0]  (N=4096, 2)
      ref = ref_catheter_skeleton[-1]  (M=32768, 2)  (the [::-1] flip in the
      reference is a permutation -> invariant for chamfer, ignored here)
      loss = sum_n min_m ||pts_n - ref_m|| + sum_m min_n ||pts_n - ref_m||

Strategy (M-sharded across 8 cores, 4096 ref points per core):
  - d2[m, n] is produced directly by a K=24 augmented matmul: each fp32
    coordinate is split host-side into 3 exact bf16 lanes (hi/mid/lo) and the
    squared norms into 4 lanes, so the bf16 PE computes fp32-grade d2 at full
    bf16 throughput (matmul cost is free-dim bound, K-independent).
  - Per (128m x 2048n) PSUM tile: ScalarE evacuates fp32 PSUM -> bf16 SBUF
    with a folded scale=-1 (so every "min" below becomes a "max" -- needed
    because the cross-partition reduce only supports max); VectorE does a
    bf16 max-tree over n (col-min, per-m) and a running bf16 tensor_tensor
    max across m-tiles (row-min, per-n) at 2x mode.
  - Row-min needs a cross-partition reduce at the end: gpsimd
    partition_all_reduce(max) over the two (128, 2048) running tiles.
    (A PE-transpose epilogue compiles + simulates but crashes TRN2 hardware
    with NRT_EXEC_UNIT_UNRECOVERABLE, so it is avoided.)
  - min(sqrt(x)) == sqrt(min(x)): sqrt runs on the host over the 36K reduced
    values only.
  - Outputs per core: 4096 complete col-mins (-d2) + 4096 partial row-mins
    (-d2); host negates, takes min across cores for rows, then sqrt+sum.
"""

import numpy as np
import ml_dtypes

BF16 = ml_dtypes.bfloat16

_N = 4096      # render points (full on every core)
_M = 32768     # total ref points
_CORES = 8
_MLOC = _M // _CORES   # 4096 ref points per core
_MT = _MLOC // 128     # 32 m-tiles
_NH = 2                # n halves
_HF = _N // _NH        # 2048 free elements per half
_K = 24                # augmented contraction lanes

# Lane pairing spec: (ref_component, pts_component). Components are
# ('x'|'y', split_idx), ('c', split_idx) or ('one',). The pts-side x/y lanes
# carry a folded factor of -2 (exact in bf16). Large-magnitude lanes first so
# the PSUM running sum cancels early (better fp32 accumulation error).
_SPEC = (
    [(("x", 0), ("x", 0)), (("c", 0), ("one",)), (("y", 0), ("y", 0)), (("one",), ("c", 0))]
    + [(("x", i), ("x", j)) for i, j in
       [(0, 1), (1, 0), (1, 1), (0, 2), (2, 0), (1, 2), (2, 1)]]
    + [(("y", i), ("y", j)) for i, j in
       [(0, 1), (1, 0), (1, 1), (0, 2), (2, 0), (1, 2), (2, 1)]]
    + [(("c", i), ("one",)) for i in (1, 2, 3)]
    + [(("one",), ("c", i)) for i in (1, 2, 3)]
)
assert len(_SPEC) == _K


def _split(v64, parts):
    """Split float64 vector into `parts` bf16 planes summing to ~v (exact
    residual splitting: plane i holds the leading bits of the remainder)."""
    out = []
    r = v64.copy()
    for _ in range(parts):
        h = r.astype(BF16)
        out.append(h)
        r = r - h.astype(np.float64)
    return out


def _components(xy):
    """xy: (n, 2) float -> dict of named bf16 component vectors."""
    x = xy[:, 0].astype(np.float64)
    y = xy[:, 1].astype(np.float64)
    comp = {}
    for name, v in (("x", x), ("y", y)):
        for i, p in enumerate(_split(v, 3)):
            comp[(name, i)] = p
    c = x * x + y * y
    for i, p in enumerate(_split(c, 4)):
        comp[("c", i)] = p
    comp[("one",)] = np.ones(len(x), BF16)
    return comp


def _lanes(xy, side):
    """Build the (K, n) bf16 lane matrix for one side ('ref' or 'pts')."""
    comp = _components(xy)
    rows = []
    for ref_c, pts_c in _SPEC:
        key = ref_c if side == "ref" else pts_c
        v = comp[key]
        if side == "pts" and key[0] in ("x", "y"):
            v = (-2.0 * v.astype(np.float64)).astype(BF16)  # exact: -2 * bf16
        rows.append(v)
    return np.stack(rows).astype(BF16)


def _build_program(reps=1):
    """Build + compile the per-core Bass program (identical on all cores)."""
    from contextlib import ExitStack
    import concourse.tile as tile
    from concourse import bacc, mybir
    from concourse import bass_isa

    f32 = mybir.dt.float32
    bf = mybir.dt.bfloat16
    MAX = mybir.AluOpType.max
    X = mybir.AxisListType.X

    nc = bacc.Bacc("TRN2", target_bir_lowering=False, debug=False,
                   num_devices=_CORES)
    lhsT_d = nc.dram_tensor("lhsT", [_K, _MLOC], bf, kind="ExternalInput").ap()
    rhs_d = nc.dram_tensor("rhs", [_K, _N], bf, kind="ExternalInput").ap()
    col_d = nc.dram_tensor("colmin", [128, _MT], f32, kind="ExternalOutput").ap()
    row_d = nc.dram_tensor("rowmin", [_NH, _HF], f32, kind="ExternalOutput").ap()

    with tile.TileContext(nc) as tc, ExitStack() as ctx:
        const = ctx.enter_context(tc.tile_pool(name="const", bufs=1))
        lh_sb = const.tile([_K, _MLOC], bf, tag="lh")
        nc.sync.dma_start(lh_sb[:], lhsT_d)
        rh_sb = const.tile([_K, _N], bf, tag="rh")
        nc.sync.dma_start(rh_sb[:], rhs_d)

        persist = ctx.enter_context(tc.tile_pool(name="persist", bufs=1))
        rowrun = [persist.tile([128, _HF], bf, tag=f"rowrun{h}",
                               name=f"rowrun{h}") for h in range(_NH)]
        colpart = persist.tile([128, _MT * _NH], f32, tag="colpart")
        colfin = persist.tile([128, _MT], f32, tag="colfin")
        allred = [persist.tile([128, _HF], f32, tag=f"allred{h}",
                               name=f"allred{h}") for h in range(_NH)]

        from contextlib import nullcontext

        def body():
            # ---- main sweep: -d2 tiles + col max-tree + row running max
            with tc.tile_pool(name="psum", bufs=2, space="PSUM") as psum_pool, \
                 tc.tile_pool(name="evac", bufs=3) as evac_pool, \
                 tc.tile_pool(name="tree", bufs=2) as tree_pool:
                for t in range(_MT):
                    for h in range(_NH):
                        pt = psum_pool.tile([128, _HF], f32, tag="pt")
                        for b in range(4):
                            nc.tensor.matmul(
                                pt[:, b * 512:(b + 1) * 512],
                                lh_sb[:, t * 128:(t + 1) * 128],
                                rh_sb[:, (h * 4 + b) * 512:(h * 4 + b + 1) * 512],
                                start=True, stop=True)
                        ev = evac_pool.tile([128, _HF], bf, tag="ev")
                        nc.scalar.mul(ev[:], pt[:], -1.0)   # ev = -d2 (bf16)
                        # row-min: running max of -d2 (bf16 TT, 2x mode)
                        if t == 0:
                            nc.vector.tensor_copy(rowrun[h][:], ev[:])
                        else:
                            nc.vector.tensor_tensor(
                                rowrun[h][:], ev[:], rowrun[h][:], MAX)
                        # col-min: max tree over the free axis
                        a1 = tree_pool.tile([128, 1024], bf, tag="t1")
                        nc.vector.tensor_tensor(
                            a1[:], ev[:, 0:1024], ev[:, 1024:2048], MAX)
                        a2 = tree_pool.tile([128, 512], bf, tag="t2")
                        nc.vector.tensor_tensor(
                            a2[:], a1[:, 0:512], a1[:, 512:1024], MAX)
                        a3 = tree_pool.tile([128, 256], bf, tag="t3")
                        nc.vector.tensor_tensor(
                            a3[:], a2[:, 0:256], a2[:, 256:512], MAX)
                        a4 = tree_pool.tile([128, 128], bf, tag="t4")
                        nc.vector.tensor_tensor(
                            a4[:], a3[:, 0:128], a3[:, 128:256], MAX)
                        nc.vector.tensor_reduce(
                            colpart[:, t * _NH + h: t * _NH + h + 1], a4[:],
                            axis=X, op=MAX)
                # fold the two n-halves of each m-tile's col-min
                nc.vector.tensor_reduce(
                    colfin[:], colpart[:].rearrange("p (t h) -> p t h", h=_NH),
                    axis=X, op=MAX)

            # ---- row-min cross-partition reduce on GpSimd (max of -d2)
            for h in range(_NH):
                nc.gpsimd.partition_all_reduce(
                    allred[h][:], rowrun[h][:], channels=128,
                    reduce_op=bass_isa.ReduceOp.max)

        if reps == 1:
            body()
        else:
            with tc.For_i(0, reps, 1):
                body()

        nc.sync.dma_start(col_d[:], colfin[:])
        for h in range(_NH):
            nc.sync.dma_start(row_d[h:h + 1, :], allred[h][0:1, :])

    nc.compile()
    return nc


_CACHE = {}


def _get_program(reps=1):
    if reps not in _CACHE:
        _CACHE[reps] = _build_program(reps)
    return _CACHE[reps]


def _make_in_maps(img_render_points, ref_catheter_skeleton):
    pts = np.asarray(img_render_points)[0].reshape(-1, 2)      # (4096, 2)
    ref = np.asarray(ref_catheter_skeleton)[-1]                # (32768, 2)
    rhs = np.ascontiguousarray(_lanes(pts, "pts"))             # (K, 4096)
    in_maps = []
    for c in range(_CORES):
        shard = ref[c * _MLOC:(c + 1) * _MLOC]
        in_maps.append({
            "lhsT": np.ascontiguousarray(_lanes(shard, "ref")),
            "rhs": rhs,
        })
    return in_maps


def _combine(results):
    """results: list of 8 {'colmin': (128, MT), 'rowmin': (NH, HF)} of -d2."""
    col_d2 = np.concatenate(
        [-r["colmin"].astype(np.float64).ravel() for r in results])
    row_d2 = np.min(
        np.stack([-r["rowmin"].astype(np.float64) for r in results]), axis=0)
    total = (np.sqrt(np.maximum(col_d2, 1e-12)).sum()
             + np.sqrt(np.maximum(row_d2, 1e-12)).sum())
    return np.float32(total)


def kernel(img_render_points, ref_catheter_skeleton):
    from concourse.bass_utils import run_bass_kernel_spmd
    nc = _get_program()
    in_maps = _make_in_maps(img_render_points, ref_catheter_skeleton)
    res = run_bass_kernel_spmd(nc, in_maps, core_ids=list(range(_CORES)))
    return _combine(res.results)


# revision 11
# speedup vs baseline: 1.1697x; 1.1697x over previous
"""Boundary-point Chamfer loss on 8 Trainium2 NeuronCores.

Math: pts = img_render_points[0]  (N=4096, 2)
      ref = ref_catheter_skeleton[-1]  (M=32768, 2)  (the [::-1] flip in the
      reference is a permutation -> invariant for chamfer, ignored here)
      loss = sum_n min_m ||pts_n - ref_m|| + sum_m min_n ||pts_n - ref_m||

Strategy (M-sharded across 8 cores, 4096 ref points per core):
  - d2[m, n] is produced directly by a K=24 augmented matmul: each fp32
    coordinate is split host-side into 3 exact bf16 lanes (hi/mid/lo) and the
    squared norms into 4 lanes, so the bf16 PE computes fp32-grade d2 at full
    bf16 throughput (matmul cost is free-dim bound, K-independent).
  - Per (128m x 2048n) PSUM tile: ScalarE evacuates fp32 PSUM -> bf16 SBUF
    with a folded scale=-1 (so every "min" below becomes a "max" -- needed
    because the cross-partition reduce only supports max); VectorE does a
    bf16 max-tree over n (col-min, per-m) and a running bf16 tensor_tensor
    max across m-tiles (row-min, per-n) at 2x mode.
  - Row-min needs a cross-partition reduce at the end: gpsimd
    partition_all_reduce(max) over the two (128, 2048) running tiles.
    (A PE-transpose epilogue compiles + simulates but crashes TRN2 hardware
    with NRT_EXEC_UNIT_UNRECOVERABLE, so it is avoided.)
  - min(sqrt(x)) == sqrt(min(x)): sqrt runs on the host over the 36K reduced
    values only.
  - Outputs per core: 4096 complete col-mins (-d2) + 4096 partial row-mins
    (-d2); host negates, takes min across cores for rows, then sqrt+sum.
"""

import numpy as np
import ml_dtypes

BF16 = ml_dtypes.bfloat16

_N = 4096      # render points (full on every core)
_M = 32768     # total ref points
_CORES = 8
_MLOC = _M // _CORES   # 4096 ref points per core
_MT = _MLOC // 128     # 32 m-tiles
_NH = 2                # n halves
_HF = _N // _NH        # 2048 free elements per half
_K = 24                # augmented contraction lanes

# Lane pairing spec: (ref_component, pts_component). Components are
# ('x'|'y', split_idx), ('c', split_idx) or ('one',). The pts-side x/y lanes
# carry a folded factor of -2 (exact in bf16). Large-magnitude lanes first so
# the PSUM running sum cancels early (better fp32 accumulation error).
_SPEC = (
    [(("x", 0), ("x", 0)), (("c", 0), ("one",)), (("y", 0), ("y", 0)), (("one",), ("c", 0))]
    + [(("x", i), ("x", j)) for i, j in
       [(0, 1), (1, 0), (1, 1), (0, 2), (2, 0), (1, 2), (2, 1)]]
    + [(("y", i), ("y", j)) for i, j in
       [(0, 1), (1, 0), (1, 1), (0, 2), (2, 0), (1, 2), (2, 1)]]
    + [(("c", i), ("one",)) for i in (1, 2, 3)]
    + [(("one",), ("c", i)) for i in (1, 2, 3)]
)
assert len(_SPEC) == _K


def _split(v64, parts):
    """Split float64 vector into `parts` bf16 planes summing to ~v (exact
    residual splitting: plane i holds the leading bits of the remainder)."""
    out = []
    r = v64.copy()
    for _ in range(parts):
        h = r.astype(BF16)
        out.append(h)
        r = r - h.astype(np.float64)
    return out


def _components(xy):
    """xy: (n, 2) float -> dict of named bf16 component vectors."""
    x = xy[:, 0].astype(np.float64)
    y = xy[:, 1].astype(np.float64)
    comp = {}
    for name, v in (("x", x), ("y", y)):
        for i, p in enumerate(_split(v, 3)):
            comp[(name, i)] = p
    c = x * x + y * y
    for i, p in enumerate(_split(c, 4)):
        comp[("c", i)] = p
    comp[("one",)] = np.ones(len(x), BF16)
    return comp


def _lanes(xy, side):
    """Build the (K, n) bf16 lane matrix for one side ('ref' or 'pts')."""
    comp = _components(xy)
    rows = []
    for ref_c, pts_c in _SPEC:
        key = ref_c if side == "ref" else pts_c
        v = comp[key]
        if side == "pts" and key[0] in ("x", "y"):
            v = (-2.0 * v.astype(np.float64)).astype(BF16)  # exact: -2 * bf16
        rows.append(v)
    return np.stack(rows).astype(BF16)


def _build_program(reps=1):
    """Build + compile the per-core Bass program (identical on all cores)."""
    from contextlib import ExitStack
    import concourse.tile as tile
    from concourse import bacc, mybir
    from concourse import bass_isa

    f32 = mybir.dt.float32
    bf = mybir.dt.bfloat16
    MAX = mybir.AluOpType.max
    X = mybir.AxisListType.X

    nc = bacc.Bacc("TRN2", target_bir_lowering=False, debug=False,
                   num_devices=_CORES)
    lhsT_d = nc.dram_tensor("lhsT", [_K, _MLOC], bf, kind="ExternalInput").ap()
    rhs_d = nc.dram_tensor("rhs", [_K, _N], bf, kind="ExternalInput").ap()
    col_d = nc.dram_tensor("colmin", [128, _MT], f32, kind="ExternalOutput").ap()
    row_d = nc.dram_tensor("rowmin", [_NH, _HF], f32, kind="ExternalOutput").ap()

    with tile.TileContext(nc) as tc, ExitStack() as ctx:
        const = ctx.enter_context(tc.tile_pool(name="const", bufs=1))
        lh_sb = const.tile([_K, _MLOC], bf, tag="lh")
        nc.sync.dma_start(lh_sb[:], lhsT_d)
        rh_sb = const.tile([_K, _N], bf, tag="rh")
        nc.sync.dma_start(rh_sb[:], rhs_d)

        persist = ctx.enter_context(tc.tile_pool(name="persist", bufs=1))
        rowrun = [persist.tile([128, _HF], bf, tag=f"rowrun{h}",
                               name=f"rowrun{h}") for h in range(_NH)]
        colpart = persist.tile([128, _MT * _NH], f32, tag="colpart")
        colfin = persist.tile([128, _MT], f32, tag="colfin")
        allred = [persist.tile([128, _HF], f32, tag=f"allred{h}",
                               name=f"allred{h}") for h in range(_NH)]

        from contextlib import nullcontext

        def body():
            # ---- main sweep: -d2 tiles + col max-tree + row running max
            with tc.tile_pool(name="psum", bufs=2, space="PSUM") as psum_pool, \
                 tc.tile_pool(name="evac", bufs=6) as evac_pool, \
                 tc.tile_pool(name="tree", bufs=3) as tree_pool:
                for t in range(_MT):
                    for h in range(_NH):
                        pt = psum_pool.tile([128, _HF], f32, tag="pt")
                        for b in range(4):
                            nc.tensor.matmul(
                                pt[:, b * 512:(b + 1) * 512],
                                lh_sb[:, t * 128:(t + 1) * 128],
                                rh_sb[:, (h * 4 + b) * 512:(h * 4 + b + 1) * 512],
                                start=True, stop=True)
                        ev = evac_pool.tile([128, _HF], bf, tag="ev")
                        nc.scalar.mul(ev[:], pt[:], -1.0)   # ev = -d2 (bf16)
                        # row-min: running max of -d2 (bf16 TT, 2x mode)
                        if t == 0:
                            nc.vector.tensor_copy(rowrun[h][:], ev[:])
                        else:
                            nc.vector.tensor_tensor(
                                rowrun[h][:], ev[:], rowrun[h][:], MAX)
                        # col-min: max tree over the free axis (all DVE; the
                        # 2x_1p bf16 mode makes each level ~out_size/2 cyc)
                        chunk = t * _NH + h
                        a1 = tree_pool.tile([128, 1024], bf, tag="t1")
                        nc.vector.tensor_tensor(
                            a1[:], ev[:, 0:1024], ev[:, 1024:2048], MAX)
                        a2 = tree_pool.tile([128, 512], bf, tag="t2")
                        nc.vector.tensor_tensor(
                            a2[:], a1[:, 0:512], a1[:, 512:1024], MAX)
                        a3 = tree_pool.tile([128, 256], bf, tag="t3")
                        nc.vector.tensor_tensor(
                            a3[:], a2[:, 0:256], a2[:, 256:512], MAX)
                        a4 = tree_pool.tile([128, 128], bf, tag="t4")
                        nc.vector.tensor_tensor(
                            a4[:], a3[:, 0:128], a3[:, 128:256], MAX)
                        nc.vector.tensor_reduce(
                            colpart[:, chunk: chunk + 1], a4[:],
                            axis=X, op=MAX)
                # fold the two n-halves of each m-tile's col-min
                nc.vector.tensor_reduce(
                    colfin[:], colpart[:].rearrange("p (t h) -> p t h", h=_NH),
                    axis=X, op=MAX)

            # ---- row-min cross-partition reduce on GpSimd (max of -d2)
            for h in range(_NH):
                nc.gpsimd.partition_all_reduce(
                    allred[h][:], rowrun[h][:], channels=128,
                    reduce_op=bass_isa.ReduceOp.max)

        if reps == 1:
            body()
        else:
            with tc.For_i(0, reps, 1):
                body()

        nc.sync.dma_start(col_d[:], colfin[:])
        for h in range(_NH):
            nc.sync.dma_start(row_d[h:h + 1, :], allred[h][0:1, :])

    nc.compile()
    return nc


_CACHE = {}


def _get_program(reps=1):
    if reps not in _CACHE:
        _CACHE[reps] = _build_program(reps)
    return _CACHE[reps]


def _make_in_maps(img_render_points, ref_catheter_skeleton):
    pts = np.asarray(img_render_points)[0].reshape(-1, 2)      # (4096, 2)
    ref = np.asarray(ref_catheter_skeleton)[-1]                # (32768, 2)
    rhs = np.ascontiguousarray(_lanes(pts, "pts"))             # (K, 4096)
    in_maps = []
    for c in range(_CORES):
        shard = ref[c * _MLOC:(c + 1) * _MLOC]
        in_maps.append({
            "lhsT": np.ascontiguousarray(_lanes(shard, "ref")),
            "rhs": rhs,
        })
    return in_maps


def _combine(results):
    """results: list of 8 {'colmin': (128, MT), 'rowmin': (NH, HF)} of -d2."""
    col_d2 = np.concatenate(
        [-r["colmin"].astype(np.float64).ravel() for r in results])
    row_d2 = np.min(
        np.stack([-r["rowmin"].astype(np.float64) for r in results]), axis=0)
    total = (np.sqrt(np.maximum(col_d2, 1e-12)).sum()
             + np.sqrt(np.maximum(row_d2, 1e-12)).sum())
    return np.float32(total)


def kernel(img_render_points, ref_catheter_skeleton):
    from concourse.bass_utils import run_bass_kernel_spmd
    nc = _get_program()
    in_maps = _make_in_maps(img_render_points, ref_catheter_skeleton)
    res = run_bass_kernel_spmd(nc, in_maps, core_ids=list(range(_CORES)))
    return _combine(res.results)
